# revision 17
# baseline (speedup 1.0000x reference)
"""Trainium2 Bass kernel for a dense transformer encoder layer.

Problem: B=4, S=2048, D=768, H=12 heads (DH=64), FFN 3072, fp32 I/O.

Sharding (no collectives): 8 cores = (batch b, sequence half) pairs.
Each core computes the full layer for its 1024 query rows; K/V projections
for the full 2048-row sequence of its batch are duplicated across the two
cores sharing a batch (cheaper than collectives here).

Layout strategy: all activations are kept feature-major ("xT" = [D, S]) so
every matmul uses native weight slices as the stationary operand and
feature-major activations as the moving operand; the attention core runs
with scoresT = [keys, q] so no on-chip transposes are ever needed. Inputs
are transposed/staged host-side (layout prep is part of sharding).

v2 performance structure (all-bf16 matmuls; measured DoubleRow fp8 is
slower than bf16 on this silicon so fp8 is not used):
 - Attention inner loop is software-pipelined with a lookahead of 2:
   program order per kc is [scores(kc) -> exp(kc) -> ctx/den(kc-2)] so the
   exp of tile kc runs on ACT/DVE while the PE does two iterations of
   other matmul work; the PE never waits on exp.
 - Softmax exp alternates between the scalar engine (table exp) and the
   vector engine (Schraudolph bit-trick exp producing bf16 bits via an
   int16 tensor_scalar), splitting the 25M-element exp load across two
   engines. Softmax denominators use the same e values they normalize, so
   the ~3% Schraudolph error cancels to ~point-wise noise (measured
   end-to-end rel err ~2e-3, gate is 2e-2).
 - FFN weights and all activation tensors are bf16 (f32 accumulation in
   PSUM); residual + LN stay f32/f32r.
 - LN mean/var for both query blocks accumulate in one PSUM bank at
   partitions 0/32/64/96 so all four reductions run col-strip concurrent.
 - Softmax denominators accumulate per-qc in one PSUM bank at partitions
   j (head A) and 32+j (head B).
"""
from contextlib import ExitStack

import numpy as np
import ml_dtypes

import concourse.bass as bass
import concourse.tile as tile
from concourse import bacc, mybir
from concourse.bass_utils import run_bass_kernel_spmd

FR = mybir.dt.float32r
F32 = mybir.dt.float32
BF = mybir.dt.bfloat16
I16 = mybir.dt.int16
AF = mybir.ActivationFunctionType
OP = mybir.AluOpType

B, S, D, H = 4, 2048, 768, 12
DH, DF = 64, 3072
SQ = 1024            # query rows per core
NK = D // 128        # 6 feature chunks
NF = DF // 128       # 24 ffn chunks
KC = S // 128        # 16 key chunks
NQ = SQ // 512       # 2 query column chunks
HP = H // 2          # 6 head pairs
NT = 8               # FFN weight slices
MF = 3               # dF 128-chunks per slice
DT = DF // NT        # 384 cols per W1 slice
N_CORES = 8
SCALE = 1.0 / 8.0    # 1/sqrt(DH)
EPS = 1e-5

GELU_FUNC = AF.Gelu
PHASES = "ABCD"   # ablation knob: subset of phases to emit
B_DEN = True      # emit den matmuls
B_CTX = True      # emit ctx matmuls
B_EXP = "split"   # "split" | "alt" | "act" | "dve" | "skip" (diagnostics)
B_DEFER = True    # defer softmax finalize into next head-pair's loop
B_CONSTW = False  # diagnostic: constant stationary slices (breaks math)

# Schraudolph exp producing bf16 bits through an int16 write:
#   bf16_bits(e^x) ~= int16(x * 2^7*log2(e) + (127 - 0.0430) * 2^7)
_LOG2E = 1.4426950408889634
A16 = (2.0 ** 7) * _LOG2E * SCALE
B16 = float(127 * 2 ** 7 - 0.043 * 2 ** 7) + 0.5  # +0.5 centers truncation

# bias pack layout (columns in "sp" [128, 828])
_BQ, _BK, _BO, _B2, _LNG, _LNB, _B1, _BV = 0, 6, 12, 18, 24, 30, 36, 60


def _body(nc, tc, io):
    xqb_d, xqf_d, xk_d, xv_d = io["xqb"], io["xqf"], io["xk"], io["xv"]
    wq_d, wk_d, wv_d, wo_d = io["wq"], io["wk"], io["wv"], io["wo"]
    w1_d, w2_d, sp_d = io["w1"], io["w2"], io["sp"]
    ones_fr_d, ones_bf_d, out_d = io["ones_fr"], io["ones_bf"], io["out"]

    r6 = lambda ap: ap.rearrange("(c p) s -> p c s", p=128)

    with ExitStack() as ctx:
        Po = lambda **kw: ctx.enter_context(tc.tile_pool(**kw))
        const = Po(name="const", bufs=1)
        sb = Po(name="sb", bufs=1)

        sp = const.tile([128, 828], F32)
        nc.sync.dma_start(out=sp[:], in_=sp_d)
        ones_fr = const.tile([128, 128], FR)
        nc.sync.dma_start(out=ones_fr[:], in_=ones_fr_d)
        ones_bf = const.tile([128, 1], BF)
        nc.sync.dma_start(out=ones_bf[:], in_=ones_bf_d)
        sel = const.tile([128, 128], FR)
        nc.sync.dma_start(out=sel[:], in_=io["sel"])
        bias = lambda idx, j: sp[:, idx + j : idx + j + 1]

        # shared weight slots: 4 x 9KB bf16
        def wtile(name, ncols):
            return sb.tile([128, ncols], BF, tag="w", bufs=4, name=name)

        wk = wtile("wk", NK * D)
        nc.sync.dma_start(out=wk[:].rearrange("p (c m) -> p c m", m=D), in_=r6(wk_d))
        wq = wtile("wq", NK * D)
        nc.sync.dma_start(out=wq[:].rearrange("p (c m) -> p c m", m=D), in_=r6(wq_d))
        wv = wtile("wv", NK * D)
        nc.sync.dma_start(out=wv[:].rearrange("p (c m) -> p c m", m=D), in_=r6(wv_d))

        e_const = None
        if B_EXP == "skip":
            e_const = const.tile([128, 1024], BF)
            nc.gpsimd.memset(e_const[:, :], 0.25)
        # persistent activations (tag overlays: kpT->x_sb, vp->hT)
        kpT = sb.tile([128, NK * S], BF, tag="kpx", name="kpT")
        qpT = sb.tile([128, NK * SQ], BF, tag="qpT", name="qpT")
        vp = sb.tile([128, KC * D], BF, tag="vph", name="vp")
        ctxS = sb.tile([128, NK * SQ], BF, tag="ctxS", name="ctxS")

        # ---------------- phase A: projections ----------------
        with tc.tile_pool(name="pa", bufs=4, space="PSUM") as pa:
            # kpT[mc, s] = sum_kc Wk[kc,mc].T @ xk[kc, s]   (+bk)
            for sc in range(S // 512):
                xk_t = sb.tile([128, NK * 512], BF, tag="xu", bufs=3, name="xk_t")
                nc.sync.dma_start(
                    out=xk_t[:].rearrange("p (c s) -> p c s", s=512),
                    in_=r6(xk_d)[:, :, sc * 512 : (sc + 1) * 512])
                for mc in range(NK):
                    ps = pa.tile([128, 512], F32, tag="pa", name="psk")
                    for kc in range(NK):
                        nc.tensor.matmul(
                            ps[:],
                            wk[:, kc * D + mc * 128 : kc * D + (mc + 1) * 128],
                            xk_t[:, kc * 512 : (kc + 1) * 512],
                            start=(kc == 0), stop=(kc == NK - 1))
                    nc.scalar.activation(
                        kpT[:, mc * S + sc * 512 : mc * S + (sc + 1) * 512],
                        ps[:], AF.Identity, bias=bias(_BK, mc), scale=1.0)

            # qpT likewise (+bq)
            for sc in range(NQ):
                xq_t = sb.tile([128, NK * 512], BF, tag="xu", bufs=3, name="xq_t")
                nc.sync.dma_start(
                    out=xq_t[:].rearrange("p (c s) -> p c s", s=512),
                    in_=r6(xqb_d)[:, :, sc * 512 : (sc + 1) * 512])
                for mc in range(NK):
                    ps = pa.tile([128, 512], F32, tag="pa", name="psq")
                    for kc in range(NK):
                        nc.tensor.matmul(
                            ps[:],
                            wq[:, kc * D + mc * 128 : kc * D + (mc + 1) * 128],
                            xq_t[:, kc * 512 : (kc + 1) * 512],
                            start=(kc == 0), stop=(kc == NK - 1))
                    nc.scalar.activation(
                        qpT[:, mc * SQ + sc * 512 : mc * SQ + (sc + 1) * 512],
                        ps[:], AF.Identity, bias=bias(_BQ, mc), scale=1.0)

            # vp[s-chunk, d] = xv[kc, s-chunk].T @ Wv[kc, d]  (+bv)
            for sc in range(S // 512):
                xv_t = sb.tile([128, NK * 512], BF, tag="xu", bufs=3, name="xv_t")
                nc.sync.dma_start(
                    out=xv_t[:].rearrange("p (c s) -> p c s", s=512),
                    in_=r6(xv_d)[:, :, sc * 512 : (sc + 1) * 512])
                for m in range(4):
                    srow = sc * 4 + m
                    for n0, nsz in ((0, 512), (512, 256)):
                        ps = pa.tile([128, 512], F32, tag="pa", name="psv")
                        for kc in range(NK):
                            nc.tensor.matmul(
                                ps[:, :nsz],
                                xv_t[:, kc * 512 + m * 128 :
                                     kc * 512 + (m + 1) * 128],
                                wv[:, kc * D + n0 : kc * D + n0 + nsz],
                                start=(kc == 0), stop=(kc == NK - 1))
                        nc.vector.tensor_add(
                            vp[:, srow * D + n0 : srow * D + n0 + nsz],
                            ps[:, :nsz], sp[:, _BV + n0 : _BV + n0 + nsz])

        xqf_sb = sb.tile([128, NK * SQ], F32, tag="xqf", name="xqf_sb")
        nc.sync.dma_start(
            out=xqf_sb[:].rearrange("p (c s) -> p c s", s=SQ), in_=r6(xqf_d))
        if "B" not in PHASES:
            ot = sb.tile([128, 512], F32, tag="sm", bufs=2, name="oend")
            nc.vector.tensor_copy(ot[:], kpT[:, 0:512])
            nc.sync.dma_start(out=r6(out_d)[:, 0, 0:512], in_=ot[:])
            return
        # ---------------- phase B: attention ----------------
        wo = wtile("wo", NK * D)
        nc.sync.dma_start(out=wo[:].rearrange("p (c m) -> p c m", m=D), in_=r6(wo_d))

        with (tc.tile_pool(name="sc_ps", bufs=2, space="PSUM") as sc_ps,
              tc.tile_pool(name="cx_ps", bufs=2, space="PSUM") as cx_ps,
              tc.tile_pool(name="dr_ps", bufs=2, space="PSUM") as dr_ps):
            pending_fin = [None]
            for qc in range(NQ):
                for j in range(HP):
                    hA, hB = 2 * j, 2 * j + 1
                    ctx_p = cx_ps.tile([128, 512], F32, tag="cx", name="ctx_p")
                    # den accumulates at partitions 0 (head A) / 32 (head B);
                    # after r_den extraction the same bank is reused for the
                    # reciprocal broadcast (rb).
                    dn = dr_ps.tile([128, 512], F32, tag="dn", name="dnrb")
                    etile = {}

                    def issue_scores(kc, j=j, qc=qc, etile=etile):
                        psAB = sc_ps.tile([128, 1024], F32, tag="sc",
                                          name="psAB")
                        ko = 0 if B_CONSTW else kc * 128
                        nc.tensor.matmul(
                            psAB[:, 0:512],
                            kpT[0:64, j * S + ko : j * S + ko + 128],
                            qpT[0:64, j * SQ + qc * 512 : j * SQ + (qc + 1) * 512],
                            start=True, stop=True)
                        nc.tensor.matmul(
                            psAB[:, 512:1024],
                            kpT[64:128, j * S + ko : j * S + ko + 128],
                            qpT[64:128, j * SQ + qc * 512 : j * SQ + (qc + 1) * 512],
                            start=True, stop=True, skip_group_check=True)
                        e = sb.tile([128, 1024], BF, tag="ex", bufs=4, name="eAB")
                        if B_EXP == "skip":
                            etile[kc] = e_const
                            return
                        if B_EXP == "split":
                            nc.scalar.activation(e[:, 0:512], psAB[:, 0:512],
                                                 AF.Exp, scale=SCALE)
                            with nc.allow_low_precision(reason="schraudolph"):
                                nc.vector.tensor_scalar(
                                    e[:, 512:1024].bitcast(I16),
                                    psAB[:, 512:1024],
                                    float(A16), float(B16), OP.mult, OP.add)
                        else:
                            use_act = ((kc % 2 == 0) if B_EXP == "alt"
                                       else (B_EXP == "act"))
                            if use_act:
                                nc.scalar.activation(e[:], psAB[:], AF.Exp,
                                                     scale=SCALE)
                            else:
                                with nc.allow_low_precision(reason="schraudolph"):
                                    nc.vector.tensor_scalar(
                                        e[:].bitcast(I16), psAB[:],
                                        float(A16), float(B16), OP.mult, OP.add)
                        etile[kc] = e

                    def issue_ctxden(kc, j=j, ctx_p=ctx_p, dn=dn, etile=etile):
                        hA, hB = 2 * j, 2 * j + 1
                        e = etile.pop(kc)
                        if not B_CTX:
                            return
                        vo = 0 if B_CONSTW else kc * D
                        nc.tensor.matmul(
                            ctx_p[0:64, :],
                            vp[:, vo + hA * 64 : vo + hA * 64 + 64],
                            e[:, 0:512], start=(kc == 0), stop=(kc == KC - 1),
                            skip_group_check=True)
                        nc.tensor.matmul(
                            ctx_p[64:128, :],
                            vp[:, vo + hB * 64 : vo + hB * 64 + 64],
                            e[:, 512:1024], start=(kc == 0), stop=(kc == KC - 1),
                            skip_group_check=True)
                        if B_DEN:
                            nc.tensor.matmul(
                                dn[0:1, :], ones_bf[:, 0:1], e[:, 0:512],
                                start=(kc == 0), stop=(kc == KC - 1),
                                skip_group_check=True)
                            nc.tensor.matmul(
                                dn[32:33, :], ones_bf[:, 0:1],
                                e[:, 512:1024],
                                start=(kc == 0), stop=(kc == KC - 1),
                                skip_group_check=True)

                    issue_scores(0)
                    issue_scores(1)
                    for kc in range(KC):
                        if kc + 2 < KC:
                            issue_scores(kc + 2)
                        issue_ctxden(kc)
                        if kc == 3 and pending_fin[0] is not None:
                            pending_fin[0]()
                            pending_fin[0] = None


                    # softmax finalize, deferred into the next head-pair's
                    # kc loop so the rb matmul never stalls the PE stream
                    def finalize(j=j, qc=qc, ctx_p=ctx_p, dn=dn):
                        r_den = sb.tile([64, 512], F32, tag="rd", name="r_den")
                        r_rec = sb.tile([64, 512], FR, tag="rr", name="r_rec")
                        nc.gpsimd.memset(r_den[:, :], 1.0)
                        if B_DEN:
                            nc.vector.tensor_copy(r_den[0:1, :], dn[0:1, :])
                            nc.vector.tensor_copy(r_den[32:33, :], dn[32:33, :])
                        with nc.allow_low_precision(reason="softmax recip"):
                            nc.vector.reciprocal(r_rec[:, :], r_den[:, :])
                        nc.tensor.matmul(dn[:], sel[0:64, :], r_rec[:],
                                         start=True, stop=True)
                        cc = sb.tile([128, 512], F32, tag="cp", bufs=2, name="cc")
                        nc.scalar.activation(cc[:], ctx_p[:], AF.Copy, scale=1.0)
                        with nc.allow_low_precision(reason="bf16 ctx"):
                            nc.vector.tensor_mul(
                                ctxS[:, j * SQ + qc * 512 : j * SQ + (qc + 1) * 512],
                                cc[:], dn[:])
                    if B_DEFER:
                        pending_fin[0] = finalize
                    else:
                        finalize()

        if "C" not in PHASES:
            ot = sb.tile([128, 512], F32, tag="sm", bufs=2, name="oend")
            nc.vector.tensor_copy(ot[:], ctxS[:, 0:512])
            nc.sync.dma_start(out=r6(out_d)[:, 0, 0:512], in_=ot[:])
            return
        if "B" in PHASES and pending_fin[0] is not None:
            pending_fin[0]()
            pending_fin[0] = None
        # ------------ phase C1: out-proj + residual ------------
        x_sb = sb.tile([128, NK * SQ], FR, tag="kpx", name="x_sb")
        with tc.tile_pool(name="pc", bufs=2, space="PSUM") as pc:
            for qc in range(NQ):
                for mc in range(NK):
                    ps = pc.tile([128, 512], F32, tag="pc", name="pso")
                    for kc in range(NK):
                        nc.tensor.matmul(
                            ps[:],
                            wo[:, kc * D + mc * 128 : kc * D + (mc + 1) * 128],
                            ctxS[:, kc * SQ + qc * 512 : kc * SQ + (qc + 1) * 512],
                            start=(kc == 0), stop=(kc == NK - 1))
                    with nc.allow_low_precision(reason="f32r residual"):
                        nc.vector.scalar_tensor_tensor(
                            x_sb[:, mc * SQ + qc * 512 : mc * SQ + (qc + 1) * 512],
                            ps[:], bias(_BO, mc),
                            xqf_sb[:, mc * SQ + qc * 512 : mc * SQ + (qc + 1) * 512],
                            OP.add, OP.add)

        # ---------------- phase C2: LayerNorm ----------------
        hT = sb.tile([128, NK * SQ], BF, tag="vph", name="hT")
        with (tc.tile_pool(name="st_ps", bufs=4, space="PSUM") as st_ps,
              tc.tile_pool(name="ab_ps", bufs=2, space="PSUM") as ab_ps):
            mean_p, var_p = {}, {}
            for qc in range(NQ):
                mean_p[qc] = st_ps.tile([1, 512], F32, tag="st", name=f"mean{qc}")
                var_p[qc] = st_ps.tile([1, 512], F32, tag="st", name=f"var{qc}")
            for kc in range(NK):
                xsq = sb.tile([128, SQ], FR, tag="sq", bufs=2, name="xsq")
                with nc.allow_low_precision(reason="f32r x^2 for LN var"):
                    nc.vector.tensor_mul(
                        xsq[:], x_sb[:, kc * SQ : (kc + 1) * SQ],
                        x_sb[:, kc * SQ : (kc + 1) * SQ])
                for qc in range(NQ):
                    nc.tensor.matmul(
                        mean_p[qc][:], ones_fr[:, 0:1],
                        x_sb[:, kc * SQ + qc * 512 : kc * SQ + (qc + 1) * 512],
                        start=(kc == 0), stop=(kc == NK - 1),
                        skip_group_check=True)
                    nc.tensor.matmul(
                        var_p[qc][:], ones_fr[:, 0:1],
                        xsq[:, qc * 512 : (qc + 1) * 512],
                        start=(kc == 0), stop=(kc == NK - 1),
                        skip_group_check=True)

            for qc in range(NQ):
                mu = sb.tile([1, 512], F32, tag="r1", bufs=2, name="mu")
                e2 = sb.tile([1, 512], F32, tag="r2", bufs=2, name="e2")
                sd = sb.tile([1, 512], F32, tag="r3", bufs=2, name="sd")
                rs = sb.tile([1, 512], FR, tag="r4", bufs=2, name="rs")
                mrs = sb.tile([1, 512], FR, tag="r5", bufs=2, name="mrs")
                nc.vector.tensor_scalar_mul(mu[:], mean_p[qc][:], 1.0 / D)
                nc.vector.tensor_scalar_mul(e2[:], var_p[qc][:], 1.0 / D)
                nc.vector.tensor_mul(sd[:], mu[:], mu[:])
                nc.vector.tensor_sub(e2[:], e2[:], sd[:])        # variance
                nc.vector.tensor_scalar_add(e2[:], e2[:], EPS)
                nc.scalar.activation(sd[:], e2[:], AF.Sqrt)
                with nc.allow_low_precision(reason="f32r LN rows"):
                    nc.vector.reciprocal(rs[:], sd[:])
                    nc.vector.tensor_mul(mrs[:], mu[:].bitcast(FR), rs[:])
                A_p = ab_ps.tile([128, 512], F32, tag="ab", name="A_p")
                B_p = ab_ps.tile([128, 512], F32, tag="ab", name="B_p")
                nc.tensor.matmul(A_p[:], ones_fr[0:1, :], rs[:],
                                 start=True, stop=True)
                nc.tensor.matmul(B_p[:], ones_fr[0:1, :], mrs[:],
                                 start=True, stop=True)
                for kc in range(NK):
                    t1 = sb.tile([128, 512], F32, tag="sm", bufs=2, name="t1")
                    nc.vector.tensor_mul(
                        t1[:],
                        x_sb[:, kc * SQ + qc * 512 : kc * SQ + (qc + 1) * 512],
                        A_p[:])
                    t2 = sb.tile([128, 512], F32, tag="sm", bufs=2, name="t2")
                    nc.vector.tensor_sub(t2[:], t1[:], B_p[:])
                    with nc.allow_low_precision(reason="bf16 hT"):
                        nc.scalar.activation(
                            hT[:, kc * SQ + qc * 512 : kc * SQ + (qc + 1) * 512],
                            t2[:], AF.Identity,
                            bias=bias(_LNB, kc), scale=bias(_LNG, kc))

        if "D" not in PHASES:
            ot = sb.tile([128, 512], F32, tag="sm", bufs=2, name="oend")
            with nc.allow_low_precision(reason="abl"):
                nc.vector.tensor_copy(ot[:], hT[:, 0:512])
            nc.sync.dma_start(out=r6(out_d)[:, 0, 0:512], in_=ot[:])
            return
        # ---------------- phase D: FFN ----------------
        with (tc.tile_pool(name="ff_ps", bufs=6, space="PSUM") as ff_ps,
              tc.tile_pool(name="u_ps", bufs=2, space="PSUM") as u_ps):
            for sc in range(NQ):
                ffp = [ff_ps.tile([128, 512], F32, tag="ff", name=f"ffp{i}")
                       for i in range(NK)]
                for t in range(NT):
                    w1t = wtile(f"w1_{sc}_{t}", NK * DT)
                    nc.sync.dma_start(
                        out=w1t[:].rearrange("p (c m) -> p c m", m=DT),
                        in_=w1_d.rearrange("(c p) (t m) -> p c t m",
                                           p=128, t=NT)[:, :, t, :])
                    w2t = wtile(f"w2_{sc}_{t}", MF * D)
                    nc.sync.dma_start(
                        out=w2t[:].rearrange("p (c m) -> p c m", m=D),
                        in_=w2_d.rearrange("(t c p) m -> p t c m",
                                           p=128, c=MF)[:, t])
                    ut = sb.tile([128, MF * 512], BF, tag="xu", bufs=3, name="ut")
                    for mf in range(MF):
                        up = u_ps.tile([128, 512], F32, tag="up", name="up")
                        for kc in range(NK):
                            nc.tensor.matmul(
                                up[:],
                                w1t[:, kc * DT + mf * 128 : kc * DT + (mf + 1) * 128],
                                hT[:, kc * SQ + sc * 512 : kc * SQ + (sc + 1) * 512],
                                start=(kc == 0), stop=(kc == NK - 1))
                        with nc.allow_low_precision(reason="bf16 u"):
                            nc.scalar.activation(
                                ut[:, mf * 512 : (mf + 1) * 512], up[:],
                                GELU_FUNC, bias=bias(_B1, t * MF + mf),
                                scale=1.0)
                    for md in range(NK):
                        for c in range(MF):
                            nc.tensor.matmul(
                                ffp[md][:],
                                w2t[:, c * D + md * 128 : c * D + (md + 1) * 128],
                                ut[:, c * 512 : (c + 1) * 512],
                                start=(t == 0 and c == 0),
                                stop=(t == NT - 1 and c == MF - 1),
                                skip_group_check=True)
                for md in range(NK):
                    ot = sb.tile([128, 512], F32, tag="sm", bufs=2, name="ot")
                    nc.scalar.activation(ot[:], ffp[md][:], AF.Identity,
                                         bias=bias(_B2, md), scale=1.0)
                    nc.sync.dma_start(
                        out=r6(out_d)[:, md, sc * 512 : (sc + 1) * 512],
                        in_=ot[:])


def _build(reps=1):
    nc = bacc.Bacc("TRN2", target_bir_lowering=False, debug=False,
                   num_devices=N_CORES)
    io = {
        "xqb": nc.dram_tensor("xqb", [D, SQ], BF, kind="ExternalInput").ap(),
        "xqf": nc.dram_tensor("xqf", [D, SQ], F32, kind="ExternalInput").ap(),
        "xk": nc.dram_tensor("xk", [D, S], BF, kind="ExternalInput").ap(),
        "xv": nc.dram_tensor("xv", [D, S], BF, kind="ExternalInput").ap(),
        "wq": nc.dram_tensor("wq", [D, D], BF, kind="ExternalInput").ap(),
        "wk": nc.dram_tensor("wk", [D, D], BF, kind="ExternalInput").ap(),
        "wv": nc.dram_tensor("wv", [D, D], BF, kind="ExternalInput").ap(),
        "wo": nc.dram_tensor("wo", [D, D], BF, kind="ExternalInput").ap(),
        "w1": nc.dram_tensor("w1", [D, DF], BF, kind="ExternalInput").ap(),
        "w2": nc.dram_tensor("w2", [DF, D], BF, kind="ExternalInput").ap(),
        "sp": nc.dram_tensor("sp", [128, 828], F32, kind="ExternalInput").ap(),
        "ones_fr": nc.dram_tensor("ones_fr", [128, 128], FR,
                                  kind="ExternalInput").ap(),
        "ones_bf": nc.dram_tensor("ones_bf", [128, 1], BF,
                                  kind="ExternalInput").ap(),
        "sel": nc.dram_tensor("sel", [128, 128], FR, kind="ExternalInput").ap(),
        "out": nc.dram_tensor("out", [D, SQ], F32, kind="ExternalOutput").ap(),
    }
    with tile.TileContext(nc) as tc:
        if reps == 1:
            _body(nc, tc, io)
        else:
            with tc.For_i(0, reps, 1):
                _body(nc, tc, io)
    nc.compile()
    return nc


_NC = None


def _get_nc():
    global _NC
    if _NC is None:
        _NC = _build()
    return _NC


def _sel_matrix():
    sel = np.zeros((128, 128), np.float32)
    sel[0, 0:64] = 1.0
    sel[32, 64:128] = 1.0
    return sel


def make_in_maps(inputs):
    """Shard + lay out the full inputs for the 8 cores (numpy only)."""
    f = lambda k: np.asarray(inputs[k], np.float32)
    bf = lambda a: np.ascontiguousarray(a).astype(ml_dtypes.bfloat16)
    Q, K, V = f("Q"), f("K"), f("V")
    sp = np.zeros((128, 828), np.float32)
    for idx, key in ((_BQ, "bq"), (_BK, "bk"), (_BO, "bo"), (_B2, "b2"),
                     (_LNG, "ln_g"), (_LNB, "ln_b")):
        sp[:, idx : idx + NK] = f(key).reshape(NK, 128).T
    sp[:, _B1 : _B1 + NF] = f("b1").reshape(NF, 128).T
    sp[:, _BV : _BV + D] = np.broadcast_to(f("bv"), (128, D))
    shared = {
        "wq": bf(f("Wq")), "wk": bf(f("Wk")), "wv": bf(f("Wv")),
        "wo": bf(f("Wo")), "w1": bf(f("W1")), "w2": bf(f("W2")), "sp": sp,
        "ones_fr": np.ones((128, 128), np.float32),
        "ones_bf": np.ones((128, 1), ml_dtypes.bfloat16),
        "sel": _sel_matrix(),
    }
    in_maps = []
    for c in range(N_CORES):
        b, half = divmod(c, 2)
        r0 = half * SQ
        xqf = np.ascontiguousarray(Q[b, r0 : r0 + SQ, :].T)
        in_maps.append(dict(
            shared,
            xqb=xqf.astype(ml_dtypes.bfloat16),
            xqf=xqf,
            xk=bf(K[b].T),
            xv=bf(V[b].T),
        ))
    return in_maps


def assemble(results):
    out = np.empty((B, S, D), np.float32)
    for c in range(N_CORES):
        b, half = divmod(c, 2)
        r0 = half * SQ
        out[b, r0 : r0 + SQ, :] = results[c]["out"].T
    return out


def kernel(**inputs):
    nc = _get_nc()
    res = run_bass_kernel_spmd(nc, make_in_maps(inputs), list(range(N_CORES)))
    return assemble(res.results)


# revision 19
# speedup vs baseline: 1.0122x; 1.0122x over previous
"""Trainium2 Bass kernel for a dense transformer encoder layer.

Problem: B=4, S=2048, D=768, H=12 heads (DH=64), FFN 3072, fp32 I/O.

Sharding (no collectives): 8 cores = (batch b, sequence half) pairs.
Each core computes the full layer for its 1024 query rows; K/V projections
for the full 2048-row sequence of its batch are duplicated across the two
cores sharing a batch (cheaper than collectives here).

Layout strategy: all activations are kept feature-major ("xT" = [D, S]) so
every matmul uses native weight slices as the stationary operand and
feature-major activations as the moving operand; the attention core runs
with scoresT = [keys, q] so no on-chip transposes are ever needed. Inputs
are transposed/staged host-side (layout prep is part of sharding).

v2 performance structure (all-bf16 matmuls; measured DoubleRow fp8 is
slower than bf16 on this silicon so fp8 is not used):
 - Attention inner loop is software-pipelined with a lookahead of 2:
   program order per kc is [scores(kc) -> exp(kc) -> ctx/den(kc-2)] so the
   exp of tile kc runs on ACT/DVE while the PE does two iterations of
   other matmul work; the PE never waits on exp.
 - Softmax exp alternates between the scalar engine (table exp) and the
   vector engine (Schraudolph bit-trick exp producing bf16 bits via an
   int16 tensor_scalar), splitting the 25M-element exp load across two
   engines. Softmax denominators use the same e values they normalize, so
   the ~3% Schraudolph error cancels to ~point-wise noise (measured
   end-to-end rel err ~2e-3, gate is 2e-2).
 - FFN weights and all activation tensors are bf16 (f32 accumulation in
   PSUM); residual + LN stay f32/f32r.
 - LN mean/var for both query blocks accumulate in one PSUM bank at
   partitions 0/32/64/96 so all four reductions run col-strip concurrent.
 - Softmax denominators accumulate per-qc in one PSUM bank at partitions
   j (head A) and 32+j (head B).
"""
from contextlib import ExitStack

import numpy as np
import ml_dtypes

import concourse.bass as bass
import concourse.tile as tile
from concourse import bacc, mybir
from concourse.bass_utils import run_bass_kernel_spmd

FR = mybir.dt.float32r
F32 = mybir.dt.float32
BF = mybir.dt.bfloat16
I16 = mybir.dt.int16
AF = mybir.ActivationFunctionType
OP = mybir.AluOpType

B, S, D, H = 4, 2048, 768, 12
DH, DF = 64, 3072
SQ = 1024            # query rows per core
NK = D // 128        # 6 feature chunks
NF = DF // 128       # 24 ffn chunks
KC = S // 128        # 16 key chunks
NQ = SQ // 512       # 2 query column chunks
HP = H // 2          # 6 head pairs
NT = 8               # FFN weight slices
MF = 3               # dF 128-chunks per slice
DT = DF // NT        # 384 cols per W1 slice
N_CORES = 8
SCALE = 1.0 / 8.0    # 1/sqrt(DH)
EPS = 1e-5

GELU_FUNC = AF.Gelu
PHASES = "ABCD"   # ablation knob: subset of phases to emit
B_DEN = True      # emit den matmuls
B_CTX = True      # emit ctx matmuls
B_EXP = "split"   # "split" | "alt" | "act" | "dve" | "skip" (diagnostics)
B_DEFER = True    # defer softmax finalize into next head-pair's loop
B_CONSTW = False  # diagnostic: constant stationary slices (breaks math)

# Schraudolph exp producing bf16 bits through an int16 write:
#   bf16_bits(e^x) ~= int16(x * 2^7*log2(e) + (127 - 0.0430) * 2^7)
_LOG2E = 1.4426950408889634
A16 = (2.0 ** 7) * _LOG2E * SCALE
B16 = float(127 * 2 ** 7 - 0.043 * 2 ** 7) + 0.5  # +0.5 centers truncation

# bias pack layout (columns in "sp" [128, 828])
_BQ, _BK, _BO, _B2, _LNG, _LNB, _B1, _BV = 0, 6, 12, 18, 24, 30, 36, 60


def _body(nc, tc, io):
    xqb_d, xqf_d, xk_d, xv_d = io["xqb"], io["xqf"], io["xk"], io["xv"]
    wq_d, wk_d, wv_d, wo_d = io["wq"], io["wk"], io["wv"], io["wo"]
    w1_d, w2_d, sp_d = io["w1"], io["w2"], io["sp"]
    ones_fr_d, ones_bf_d, out_d = io["ones_fr"], io["ones_bf"], io["out"]

    r6 = lambda ap: ap.rearrange("(c p) s -> p c s", p=128)

    with ExitStack() as ctx:
        Po = lambda **kw: ctx.enter_context(tc.tile_pool(**kw))
        const = Po(name="const", bufs=1)
        sb = Po(name="sb", bufs=1)

        sp = const.tile([128, 828], F32)
        nc.sync.dma_start(out=sp[:], in_=sp_d)
        ones_fr = const.tile([128, 128], FR)
        nc.sync.dma_start(out=ones_fr[:], in_=ones_fr_d)
        ones_bf = const.tile([128, 1], BF)
        nc.sync.dma_start(out=ones_bf[:], in_=ones_bf_d)
        sel = const.tile([128, 128], FR)
        nc.sync.dma_start(out=sel[:], in_=io["sel"])
        bias = lambda idx, j: sp[:, idx + j : idx + j + 1]

        # shared weight slots: 4 x 9KB bf16
        def wtile(name, ncols):
            return sb.tile([128, ncols], BF, tag="w", bufs=4, name=name)

        wk = wtile("wk", NK * D)
        nc.sync.dma_start(out=wk[:].rearrange("p (c m) -> p c m", m=D), in_=r6(wk_d))
        wq = wtile("wq", NK * D)
        nc.sync.dma_start(out=wq[:].rearrange("p (c m) -> p c m", m=D), in_=r6(wq_d))
        wv = wtile("wv", NK * D)
        nc.sync.dma_start(out=wv[:].rearrange("p (c m) -> p c m", m=D), in_=r6(wv_d))

        e_const = None
        if B_EXP == "skip":
            e_const = const.tile([128, 1024], BF)
            nc.gpsimd.memset(e_const[:, :], 0.25)
        # persistent activations (tag overlays: kpT->x_sb, vp->hT)
        kpT = sb.tile([128, NK * S], BF, tag="kpx", name="kpT")
        qpT = sb.tile([128, NK * SQ], BF, tag="qpT", name="qpT")
        vp = sb.tile([128, KC * D], BF, tag="vph", name="vp")
        ctxS = sb.tile([128, NK * SQ], BF, tag="ctxS", name="ctxS")

        # ---------------- phase A: projections ----------------
        with tc.tile_pool(name="pa", bufs=4, space="PSUM") as pa:
            # kpT[mc, s] = sum_kc Wk[kc,mc].T @ xk[kc, s]   (+bk)
            for sc in range(S // 512):
                xk_t = sb.tile([128, NK * 512], BF, tag="xu", bufs=3, name="xk_t")
                nc.sync.dma_start(
                    out=xk_t[:].rearrange("p (c s) -> p c s", s=512),
                    in_=r6(xk_d)[:, :, sc * 512 : (sc + 1) * 512])
                for mc in range(NK):
                    ps = pa.tile([128, 512], F32, tag="pa", name="psk")
                    for kc in range(NK):
                        nc.tensor.matmul(
                            ps[:],
                            wk[:, kc * D + mc * 128 : kc * D + (mc + 1) * 128],
                            xk_t[:, kc * 512 : (kc + 1) * 512],
                            start=(kc == 0), stop=(kc == NK - 1))
                    nc.scalar.activation(
                        kpT[:, mc * S + sc * 512 : mc * S + (sc + 1) * 512],
                        ps[:], AF.Identity, bias=bias(_BK, mc), scale=1.0)

            # qpT likewise (+bq)
            for sc in range(NQ):
                xq_t = sb.tile([128, NK * 512], BF, tag="xu", bufs=3, name="xq_t")
                nc.sync.dma_start(
                    out=xq_t[:].rearrange("p (c s) -> p c s", s=512),
                    in_=r6(xqb_d)[:, :, sc * 512 : (sc + 1) * 512])
                for mc in range(NK):
                    ps = pa.tile([128, 512], F32, tag="pa", name="psq")
                    for kc in range(NK):
                        nc.tensor.matmul(
                            ps[:],
                            wq[:, kc * D + mc * 128 : kc * D + (mc + 1) * 128],
                            xq_t[:, kc * 512 : (kc + 1) * 512],
                            start=(kc == 0), stop=(kc == NK - 1))
                    nc.scalar.activation(
                        qpT[:, mc * SQ + sc * 512 : mc * SQ + (sc + 1) * 512],
                        ps[:], AF.Identity, bias=bias(_BQ, mc), scale=1.0)

            # vp[s-chunk, d] = xv[kc, s-chunk].T @ Wv[kc, d]  (+bv)
            for sc in range(S // 512):
                xv_t = sb.tile([128, NK * 512], BF, tag="xu", bufs=3, name="xv_t")
                nc.sync.dma_start(
                    out=xv_t[:].rearrange("p (c s) -> p c s", s=512),
                    in_=r6(xv_d)[:, :, sc * 512 : (sc + 1) * 512])
                for m in range(4):
                    srow = sc * 4 + m
                    for n0, nsz in ((0, 512), (512, 256)):
                        ps = pa.tile([128, 512], F32, tag="pa", name="psv")
                        for kc in range(NK):
                            nc.tensor.matmul(
                                ps[:, :nsz],
                                xv_t[:, kc * 512 + m * 128 :
                                     kc * 512 + (m + 1) * 128],
                                wv[:, kc * D + n0 : kc * D + n0 + nsz],
                                start=(kc == 0), stop=(kc == NK - 1))
                        nc.vector.tensor_add(
                            vp[:, srow * D + n0 : srow * D + n0 + nsz],
                            ps[:, :nsz], sp[:, _BV + n0 : _BV + n0 + nsz])

        xqf_sb = sb.tile([128, NK * SQ], F32, tag="xqf", name="xqf_sb")
        nc.sync.dma_start(
            out=xqf_sb[:].rearrange("p (c s) -> p c s", s=SQ), in_=r6(xqf_d))
        if "B" not in PHASES:
            ot = sb.tile([128, 512], F32, tag="sm", bufs=2, name="oend")
            nc.vector.tensor_copy(ot[:], kpT[:, 0:512])
            nc.sync.dma_start(out=r6(out_d)[:, 0, 0:512], in_=ot[:])
            return
        # ---------------- phase B: attention ----------------
        wo = wtile("wo", NK * D)
        nc.sync.dma_start(out=wo[:].rearrange("p (c m) -> p c m", m=D), in_=r6(wo_d))

        with (tc.tile_pool(name="sc_ps", bufs=2, space="PSUM") as sc_ps,
              tc.tile_pool(name="cx_ps", bufs=2, space="PSUM") as cx_ps,
              tc.tile_pool(name="dr_ps", bufs=2, space="PSUM") as dr_ps):
            pending_fin = [None]
            for qc in range(NQ):
                for j in range(HP):
                    hA, hB = 2 * j, 2 * j + 1
                    ctx_p = cx_ps.tile([128, 512], F32, tag="cx", name="ctx_p")
                    # den accumulates at partitions 0 (head A) / 32 (head B);
                    # after r_den extraction the same bank is reused for the
                    # reciprocal broadcast (rb).
                    dn = dr_ps.tile([128, 512], F32, tag="dn", name="dnrb")
                    etile = {}

                    def issue_scores(kc, j=j, qc=qc, etile=etile):
                        psAB = sc_ps.tile([128, 1024], F32, tag="sc",
                                          name="psAB")
                        ko = 0 if B_CONSTW else kc * 128
                        nc.tensor.matmul(
                            psAB[:, 0:512],
                            kpT[0:64, j * S + ko : j * S + ko + 128],
                            qpT[0:64, j * SQ + qc * 512 : j * SQ + (qc + 1) * 512],
                            start=True, stop=True)
                        nc.tensor.matmul(
                            psAB[:, 512:1024],
                            kpT[64:128, j * S + ko : j * S + ko + 128],
                            qpT[64:128, j * SQ + qc * 512 : j * SQ + (qc + 1) * 512],
                            start=True, stop=True, skip_group_check=True)
                        e = sb.tile([128, 1024], BF, tag="ex", bufs=4, name="eAB")
                        if B_EXP == "skip":
                            etile[kc] = e_const
                            return
                        if B_EXP == "split":
                            nc.scalar.activation(e[:, 0:512], psAB[:, 0:512],
                                                 AF.Exp, scale=SCALE)
                            with nc.allow_low_precision(reason="schraudolph"):
                                nc.vector.tensor_scalar(
                                    e[:, 512:1024].bitcast(I16),
                                    psAB[:, 512:1024],
                                    float(A16), float(B16), OP.mult, OP.add)
                        else:
                            use_act = ((kc % 2 == 0) if B_EXP == "alt"
                                       else (B_EXP == "act"))
                            if use_act:
                                nc.scalar.activation(e[:], psAB[:], AF.Exp,
                                                     scale=SCALE)
                            else:
                                with nc.allow_low_precision(reason="schraudolph"):
                                    nc.vector.tensor_scalar(
                                        e[:].bitcast(I16), psAB[:],
                                        float(A16), float(B16), OP.mult, OP.add)
                        etile[kc] = e

                    def issue_ctxden(kc, j=j, ctx_p=ctx_p, dn=dn, etile=etile):
                        hA, hB = 2 * j, 2 * j + 1
                        e = etile.pop(kc)
                        if not B_CTX:
                            return
                        vo = 0 if B_CONSTW else kc * D
                        nc.tensor.matmul(
                            ctx_p[0:64, :],
                            vp[:, vo + hA * 64 : vo + hA * 64 + 64],
                            e[:, 0:512], start=(kc == 0), stop=(kc == KC - 1),
                            skip_group_check=True)
                        nc.tensor.matmul(
                            ctx_p[64:128, :],
                            vp[:, vo + hB * 64 : vo + hB * 64 + 64],
                            e[:, 512:1024], start=(kc == 0), stop=(kc == KC - 1),
                            skip_group_check=True)
                        if B_DEN:
                            nc.tensor.matmul(
                                dn[0:1, :], ones_bf[:, 0:1], e[:, 0:512],
                                start=(kc == 0), stop=(kc == KC - 1),
                                skip_group_check=True)
                            nc.tensor.matmul(
                                dn[32:33, :], ones_bf[:, 0:1],
                                e[:, 512:1024],
                                start=(kc == 0), stop=(kc == KC - 1),
                                skip_group_check=True)

                    issue_scores(0)
                    issue_scores(1)
                    for kc in range(KC):
                        if kc + 2 < KC:
                            issue_scores(kc + 2)
                        issue_ctxden(kc)
                        if kc == 3 and pending_fin[0] is not None:
                            pending_fin[0]()
                            pending_fin[0] = None


                    # softmax finalize, deferred into the next head-pair's
                    # kc loop so the rb matmul never stalls the PE stream
                    def finalize(j=j, qc=qc, ctx_p=ctx_p, dn=dn):
                        r_den = sb.tile([64, 512], F32, tag="rd", name="r_den")
                        r_rec = sb.tile([64, 512], FR, tag="rr", name="r_rec")
                        nc.gpsimd.memset(r_den[:, :], 1.0)
                        if B_DEN:
                            nc.vector.tensor_copy(r_den[0:1, :], dn[0:1, :])
                            nc.vector.tensor_copy(r_den[32:33, :], dn[32:33, :])
                        with nc.allow_low_precision(reason="softmax recip"):
                            nc.vector.reciprocal(r_rec[:, :], r_den[:, :])
                        nc.tensor.matmul(dn[:], sel[0:64, :], r_rec[:],
                                         start=True, stop=True)
                        cc = sb.tile([128, 512], F32, tag="cp", bufs=2, name="cc")
                        nc.scalar.activation(cc[:], ctx_p[:], AF.Copy, scale=1.0)
                        with nc.allow_low_precision(reason="bf16 ctx"):
                            nc.vector.tensor_mul(
                                ctxS[:, j * SQ + qc * 512 : j * SQ + (qc + 1) * 512],
                                cc[:], dn[:])
                    if B_DEFER:
                        pending_fin[0] = finalize
                    else:
                        finalize()

        if "C" not in PHASES:
            ot = sb.tile([128, 512], F32, tag="sm", bufs=2, name="oend")
            nc.vector.tensor_copy(ot[:], ctxS[:, 0:512])
            nc.sync.dma_start(out=r6(out_d)[:, 0, 0:512], in_=ot[:])
            return
        if "B" in PHASES and pending_fin[0] is not None:
            pending_fin[0]()
            pending_fin[0] = None
        # ------------ phase C1: out-proj + residual ------------
        x_sb = sb.tile([128, NK * SQ], FR, tag="kpx", name="x_sb")
        with tc.tile_pool(name="pc", bufs=2, space="PSUM") as pc:
            for qc in range(NQ):
                for mc in range(NK):
                    ps = pc.tile([128, 512], F32, tag="pc", name="pso")
                    for kc in range(NK):
                        nc.tensor.matmul(
                            ps[:],
                            wo[:, kc * D + mc * 128 : kc * D + (mc + 1) * 128],
                            ctxS[:, kc * SQ + qc * 512 : kc * SQ + (qc + 1) * 512],
                            start=(kc == 0), stop=(kc == NK - 1))
                    with nc.allow_low_precision(reason="f32r residual"):
                        nc.vector.scalar_tensor_tensor(
                            x_sb[:, mc * SQ + qc * 512 : mc * SQ + (qc + 1) * 512],
                            ps[:], bias(_BO, mc),
                            xqf_sb[:, mc * SQ + qc * 512 : mc * SQ + (qc + 1) * 512],
                            OP.add, OP.add)

        # ---------------- phase C2: LayerNorm ----------------
        hT = sb.tile([128, NK * SQ], BF, tag="vph", name="hT")
        with (tc.tile_pool(name="st_ps", bufs=4, space="PSUM") as st_ps,
              tc.tile_pool(name="ab_ps", bufs=2, space="PSUM") as ab_ps):
            mean_p, var_p = {}, {}
            for qc in range(NQ):
                mean_p[qc] = st_ps.tile([1, 512], F32, tag="st", name=f"mean{qc}")
                var_p[qc] = st_ps.tile([1, 512], F32, tag="st", name=f"var{qc}")
            for kc in range(NK):
                xsq = sb.tile([128, SQ], FR, tag="sq", bufs=2, name="xsq")
                with nc.allow_low_precision(reason="f32r x^2 for LN var"):
                    nc.vector.tensor_mul(
                        xsq[:], x_sb[:, kc * SQ : (kc + 1) * SQ],
                        x_sb[:, kc * SQ : (kc + 1) * SQ])
                for qc in range(NQ):
                    nc.tensor.matmul(
                        mean_p[qc][:], ones_fr[:, 0:1],
                        x_sb[:, kc * SQ + qc * 512 : kc * SQ + (qc + 1) * 512],
                        start=(kc == 0), stop=(kc == NK - 1),
                        skip_group_check=True)
                    nc.tensor.matmul(
                        var_p[qc][:], ones_fr[:, 0:1],
                        xsq[:, qc * 512 : (qc + 1) * 512],
                        start=(kc == 0), stop=(kc == NK - 1),
                        skip_group_check=True)

            for qc in range(NQ):
                mu = sb.tile([1, 512], F32, tag="r1", bufs=2, name="mu")
                e2 = sb.tile([1, 512], F32, tag="r2", bufs=2, name="e2")
                sd = sb.tile([1, 512], F32, tag="r3", bufs=2, name="sd")
                rs = sb.tile([1, 512], FR, tag="r4", bufs=2, name="rs")
                mrs = sb.tile([1, 512], FR, tag="r5", bufs=2, name="mrs")
                nc.vector.tensor_scalar_mul(mu[:], mean_p[qc][:], 1.0 / D)
                nc.vector.tensor_scalar_mul(e2[:], var_p[qc][:], 1.0 / D)
                nc.vector.tensor_mul(sd[:], mu[:], mu[:])
                nc.vector.tensor_sub(e2[:], e2[:], sd[:])        # variance
                nc.vector.tensor_scalar_add(e2[:], e2[:], EPS)
                nc.scalar.activation(sd[:], e2[:], AF.Sqrt)
                with nc.allow_low_precision(reason="f32r LN rows"):
                    nc.vector.reciprocal(rs[:], sd[:])
                    nc.vector.tensor_mul(mrs[:], mu[:].bitcast(FR), rs[:])
                A_p = ab_ps.tile([128, 512], F32, tag="ab", name="A_p")
                B_p = ab_ps.tile([128, 512], F32, tag="ab", name="B_p")
                nc.tensor.matmul(A_p[:], ones_fr[0:1, :], rs[:],
                                 start=True, stop=True)
                nc.tensor.matmul(B_p[:], ones_fr[0:1, :], mrs[:],
                                 start=True, stop=True)
                for kc in range(NK):
                    t1 = sb.tile([128, 512], F32, tag="sm", bufs=2, name="t1")
                    nc.vector.tensor_mul(
                        t1[:],
                        x_sb[:, kc * SQ + qc * 512 : kc * SQ + (qc + 1) * 512],
                        A_p[:])
                    t2 = sb.tile([128, 512], F32, tag="sm", bufs=2, name="t2")
                    nc.vector.tensor_sub(t2[:], t1[:], B_p[:])
                    with nc.allow_low_precision(reason="bf16 hT"):
                        nc.scalar.activation(
                            hT[:, kc * SQ + qc * 512 : kc * SQ + (qc + 1) * 512],
                            t2[:], AF.Identity,
                            bias=bias(_LNB, kc), scale=bias(_LNG, kc))

        if "D" not in PHASES:
            ot = sb.tile([128, 512], F32, tag="sm", bufs=2, name="oend")
            with nc.allow_low_precision(reason="abl"):
                nc.vector.tensor_copy(ot[:], hT[:, 0:512])
            nc.sync.dma_start(out=r6(out_d)[:, 0, 0:512], in_=ot[:])
            return
        # ---------------- phase D: FFN ----------------
        with (tc.tile_pool(name="ff_ps", bufs=6, space="PSUM") as ff_ps,
              tc.tile_pool(name="u_ps", bufs=2, space="PSUM") as u_ps):
            for sc in range(NQ):
                ffp = [ff_ps.tile([128, 512], F32, tag="ff", name=f"ffp{i}")
                       for i in range(NK)]
                for t in range(NT):
                    w1t = wtile(f"w1_{sc}_{t}", NK * DT)
                    nc.sync.dma_start(
                        out=w1t[:].rearrange("p (c m) -> p c m", m=DT),
                        in_=w1_d.rearrange("(c p) (t m) -> p c t m",
                                           p=128, t=NT)[:, :, t, :])
                    w2t = wtile(f"w2_{sc}_{t}", MF * D)
                    nc.sync.dma_start(
                        out=w2t[:].rearrange("p (c m) -> p c m", m=D),
                        in_=w2_d.rearrange("(t c p) m -> p t c m",
                                           p=128, c=MF)[:, t])
                    ut = sb.tile([128, MF * 512], BF, tag="xu", bufs=3, name="ut")
                    for mf in range(MF):
                        up = u_ps.tile([128, 512], F32, tag="up", name="up")
                        for kc in range(NK):
                            nc.tensor.matmul(
                                up[:],
                                w1t[:, kc * DT + mf * 128 : kc * DT + (mf + 1) * 128],
                                hT[:, kc * SQ + sc * 512 : kc * SQ + (sc + 1) * 512],
                                start=(kc == 0), stop=(kc == NK - 1))
                        with nc.allow_low_precision(reason="bf16 u"):
                            nc.scalar.activation(
                                ut[:, mf * 512 : (mf + 1) * 512], up[:],
                                GELU_FUNC, bias=bias(_B1, t * MF + mf),
                                scale=1.0)
                    for md in range(NK):
                        for c in range(MF):
                            nc.tensor.matmul(
                                ffp[md][:],
                                w2t[:, c * D + md * 128 : c * D + (md + 1) * 128],
                                ut[:, c * 512 : (c + 1) * 512],
                                start=(t == 0 and c == 0),
                                stop=(t == NT - 1 and c == MF - 1),
                                skip_group_check=True)
                for md in range(NK):
                    ot = sb.tile([128, 512], F32, tag="sm", bufs=2, name="ot")
                    nc.scalar.activation(ot[:], ffp[md][:], AF.Identity,
                                         bias=bias(_B2, md), scale=1.0)
                    nc.sync.dma_start(
                        out=r6(out_d)[:, md, sc * 512 : (sc + 1) * 512],
                        in_=ot[:])


def _build(reps=1):
    nc = bacc.Bacc("TRN2", target_bir_lowering=False, debug=False,
                   num_devices=N_CORES)
    io = {
        "xqb": nc.dram_tensor("xqb", [D, SQ], BF, kind="ExternalInput").ap(),
        "xqf": nc.dram_tensor("xqf", [D, SQ], F32, kind="ExternalInput").ap(),
        "xk": nc.dram_tensor("xk", [D, S], BF, kind="ExternalInput").ap(),
        "xv": nc.dram_tensor("xv", [D, S], BF, kind="ExternalInput").ap(),
        "wq": nc.dram_tensor("wq", [D, D], BF, kind="ExternalInput").ap(),
        "wk": nc.dram_tensor("wk", [D, D], BF, kind="ExternalInput").ap(),
        "wv": nc.dram_tensor("wv", [D, D], BF, kind="ExternalInput").ap(),
        "wo": nc.dram_tensor("wo", [D, D], BF, kind="ExternalInput").ap(),
        "w1": nc.dram_tensor("w1", [D, DF], BF, kind="ExternalInput").ap(),
        "w2": nc.dram_tensor("w2", [DF, D], BF, kind="ExternalInput").ap(),
        "sp": nc.dram_tensor("sp", [128, 828], F32, kind="ExternalInput").ap(),
        "ones_fr": nc.dram_tensor("ones_fr", [128, 128], FR,
                                  kind="ExternalInput").ap(),
        "ones_bf": nc.dram_tensor("ones_bf", [128, 1], BF,
                                  kind="ExternalInput").ap(),
        "sel": nc.dram_tensor("sel", [128, 128], FR, kind="ExternalInput").ap(),
        "out": nc.dram_tensor("out", [D, SQ], F32, kind="ExternalOutput").ap(),
    }
    with tile.TileContext(nc) as tc:
        if reps == 1:
            _body(nc, tc, io)
        else:
            with tc.For_i(0, reps, 1):
                _body(nc, tc, io)
    nc.compile()
    return nc


_NC = None


def _get_nc():
    global _NC
    if _NC is None:
        _NC = _build()
    return _NC


def _sel_matrix():
    sel = np.zeros((128, 128), np.float32)
    sel[0, 0:64] = 1.0
    sel[32, 64:128] = 1.0
    return sel


def make_in_maps(inputs):
    """Shard + lay out the full inputs for the 8 cores (numpy only)."""
    f = lambda k: np.asarray(inputs[k], np.float32)
    bf = lambda a: np.ascontiguousarray(a).astype(ml_dtypes.bfloat16)
    Q, K, V = f("Q"), f("K"), f("V")
    sp = np.zeros((128, 828), np.float32)
    for idx, key in ((_BQ, "bq"), (_BK, "bk"), (_BO, "bo"), (_B2, "b2"),
                     (_LNG, "ln_g"), (_LNB, "ln_b")):
        sp[:, idx : idx + NK] = f(key).reshape(NK, 128).T
    sp[:, _B1 : _B1 + NF] = f("b1").reshape(NF, 128).T
    sp[:, _BV : _BV + D] = np.broadcast_to(f("bv"), (128, D))
    shared = {
        "wq": bf(f("Wq")), "wk": bf(f("Wk")), "wv": bf(f("Wv")),
        "wo": bf(f("Wo")), "w1": bf(f("W1")), "w2": bf(f("W2")), "sp": sp,
        "ones_fr": np.ones((128, 128), np.float32),
        "ones_bf": np.ones((128, 1), ml_dtypes.bfloat16),
        "sel": _sel_matrix(),
    }
    in_maps = []
    for c in range(N_CORES):
        b, half = divmod(c, 2)
        r0 = half * SQ
        xqf = np.ascontiguousarray(Q[b, r0 : r0 + SQ, :].T)
        in_maps.append(dict(
            shared,
            xqb=xqf.astype(ml_dtypes.bfloat16),
            xqf=xqf,
            xk=bf(K[b].T),
            xv=bf(V[b].T),
        ))
    return in_maps


def assemble(results):
    out = np.empty((B, S, D), np.float32)
    for c in range(N_CORES):
        b, half = divmod(c, 2)
        r0 = half * SQ
        out[b, r0 : r0 + SQ, :] = results[c]["out"].T
    return out


def kernel(**inputs):
    nc = _get_nc()
    res = run_bass_kernel_spmd(nc, make_in_maps(inputs), list(range(N_CORES)))
    return assemble(res.results)


# revision 22
# speedup vs baseline: 1.1110x; 1.0976x over previous
"""Trainium2 Bass kernel for a dense transformer encoder layer.

Problem: B=4, S=2048, D=768, H=12 heads (DH=64), FFN 3072, fp32 I/O.

Sharding (no collectives): 8 cores = (batch b, sequence half) pairs.
Each core computes the full layer for its 1024 query rows; K/V projections
for the full 2048-row sequence of its batch are duplicated across the two
cores sharing a batch (cheaper than collectives here).

Layout strategy: all activations are kept feature-major ("xT" = [D, S]) so
every matmul uses native weight slices as the stationary operand and
feature-major activations as the moving operand; the attention core runs
with scoresT = [keys, q] so no on-chip transposes are ever needed. Inputs
are transposed/staged host-side (layout prep is part of sharding).

v2 performance structure (all-bf16 matmuls; measured DoubleRow fp8 is
slower than bf16 on this silicon so fp8 is not used):
 - Attention inner loop is software-pipelined with a lookahead of 2:
   program order per kc is [scores(kc) -> exp(kc) -> ctx/den(kc-2)] so the
   exp of tile kc runs on ACT/DVE while the PE does two iterations of
   other matmul work; the PE never waits on exp.
 - Softmax exp alternates between the scalar engine (table exp) and the
   vector engine (Schraudolph bit-trick exp producing bf16 bits via an
   int16 tensor_scalar), splitting the 25M-element exp load across two
   engines. Softmax denominators use the same e values they normalize, so
   the ~3% Schraudolph error cancels to ~point-wise noise (measured
   end-to-end rel err ~2e-3, gate is 2e-2).
 - FFN weights and all activation tensors are bf16 (f32 accumulation in
   PSUM); residual + LN stay f32/f32r.
 - LN mean/var for both query blocks accumulate in one PSUM bank at
   partitions 0/32/64/96 so all four reductions run col-strip concurrent.
 - Softmax denominators accumulate per-qc in one PSUM bank at partitions
   j (head A) and 32+j (head B).
"""
from contextlib import ExitStack

import numpy as np
import ml_dtypes

import concourse.bass as bass
import concourse.tile as tile
from concourse import bacc, mybir
from concourse.bass_utils import run_bass_kernel_spmd

FR = mybir.dt.float32r
F32 = mybir.dt.float32
BF = mybir.dt.bfloat16
I16 = mybir.dt.int16
AF = mybir.ActivationFunctionType
OP = mybir.AluOpType

B, S, D, H = 4, 2048, 768, 12
DH, DF = 64, 3072
SQ = 1024            # query rows per core
NK = D // 128        # 6 feature chunks
NF = DF // 128       # 24 ffn chunks
KC = S // 128        # 16 key chunks
NQ = SQ // 512       # 2 query column chunks
HP = H // 2          # 6 head pairs
NT = 8               # FFN weight slices
MF = 3               # dF 128-chunks per slice
DT = DF // NT        # 384 cols per W1 slice
N_CORES = 8
SCALE = 1.0 / 8.0    # 1/sqrt(DH)
EPS = 1e-5

GELU_FUNC = AF.Gelu
PHASES = "ABCD"   # ablation knob: subset of phases to emit
B_DEN = True      # emit den matmuls
B_CTX = True      # emit ctx matmuls
B_EXP = "split"   # "split" | "alt" | "act" | "dve" | "skip" (diagnostics)
B_DEFER = True    # defer softmax finalize into next head-pair's loop
B_CONSTW = False  # diagnostic: constant stationary slices (breaks math)

# Schraudolph exp producing bf16 bits through an int16 write:
#   bf16_bits(e^x) ~= int16(x * 2^7*log2(e) + (127 - 0.0430) * 2^7)
_LOG2E = 1.4426950408889634
A16 = (2.0 ** 7) * _LOG2E * SCALE
B16 = float(127 * 2 ** 7 - 0.043 * 2 ** 7) + 0.5  # +0.5 centers truncation

# bias pack layout (columns in "sp" [128, 828])
_BQ, _BK, _BO, _B2, _LNG, _LNB, _B1, _BV = 0, 6, 12, 18, 24, 30, 36, 60


def _body(nc, tc, io):
    xqb_d, xqf_d, xk_d, xv_d = io["xqb"], io["xqf"], io["xk"], io["xv"]
    wq_d, wk_d, wv_d, wo_d = io["wq"], io["wk"], io["wv"], io["wo"]
    w1_d, w2_d, sp_d = io["w1"], io["w2"], io["sp"]
    ones_fr_d, ones_bf_d, out_d = io["ones_fr"], io["ones_bf"], io["out"]

    r6 = lambda ap: ap.rearrange("(c p) s -> p c s", p=128)

    with ExitStack() as ctx:
        Po = lambda **kw: ctx.enter_context(tc.tile_pool(**kw))
        const = Po(name="const", bufs=1)
        sb = Po(name="sb", bufs=1)

        sp = const.tile([128, 828], F32)
        nc.sync.dma_start(out=sp[:], in_=sp_d)
        ones_fr = const.tile([128, 128], FR)
        nc.sync.dma_start(out=ones_fr[:], in_=ones_fr_d)
        ones_bf = const.tile([128, 1], BF)
        nc.sync.dma_start(out=ones_bf[:], in_=ones_bf_d)
        sel = const.tile([128, 128], FR)
        nc.sync.dma_start(out=sel[:], in_=io["sel"])
        bias = lambda idx, j: sp[:, idx + j : idx + j + 1]

        # shared weight slots: 4 x 9KB bf16
        def wtile(name, ncols):
            return sb.tile([128, ncols], BF, tag="w", bufs=4, name=name)

        wk = wtile("wk", NK * D)
        nc.sync.dma_start(out=wk[:].rearrange("p (c m) -> p c m", m=D), in_=r6(wk_d))
        wq = wtile("wq", NK * D)
        nc.sync.dma_start(out=wq[:].rearrange("p (c m) -> p c m", m=D), in_=r6(wq_d))
        wv = wtile("wv", NK * D)
        nc.sync.dma_start(out=wv[:].rearrange("p (c m) -> p c m", m=D), in_=r6(wv_d))

        e_const = None
        if B_EXP == "skip":
            e_const = const.tile([128, 512], BF)
            nc.gpsimd.memset(e_const[:, :], 0.25)
        # persistent activations (tag overlays: kpT->x_sb, vp->hT)
        kpT = sb.tile([128, NK * S], BF, tag="kpx", name="kpT")
        qpT = sb.tile([128, NK * SQ], BF, tag="qpT", name="qpT")
        vp = sb.tile([128, KC * D], BF, tag="vph", name="vp")
        ctxS = sb.tile([128, NK * SQ], BF, tag="ctxS", name="ctxS")

        # ---------------- phase A: projections ----------------
        with tc.tile_pool(name="pa", bufs=4, space="PSUM") as pa:
            # kpT[mc, s] = sum_kc Wk[kc,mc].T @ xk[kc, s]   (+bk)
            for sc in range(S // 512):
                xk_t = sb.tile([128, NK * 512], BF, tag="xu", bufs=2, name="xk_t")
                nc.sync.dma_start(
                    out=xk_t[:].rearrange("p (c s) -> p c s", s=512),
                    in_=r6(xk_d)[:, :, sc * 512 : (sc + 1) * 512])
                for mc in range(NK):
                    ps = pa.tile([128, 512], F32, tag="pa", name="psk")
                    for kc in range(NK):
                        nc.tensor.matmul(
                            ps[:],
                            wk[:, kc * D + mc * 128 : kc * D + (mc + 1) * 128],
                            xk_t[:, kc * 512 : (kc + 1) * 512],
                            start=(kc == 0), stop=(kc == NK - 1))
                    nc.scalar.activation(
                        kpT[:, mc * S + sc * 512 : mc * S + (sc + 1) * 512],
                        ps[:], AF.Identity, bias=bias(_BK, mc), scale=1.0)

            # qpT likewise (+bq)
            for sc in range(NQ):
                xq_t = sb.tile([128, NK * 512], BF, tag="xu", bufs=2, name="xq_t")
                nc.sync.dma_start(
                    out=xq_t[:].rearrange("p (c s) -> p c s", s=512),
                    in_=r6(xqb_d)[:, :, sc * 512 : (sc + 1) * 512])
                for mc in range(NK):
                    ps = pa.tile([128, 512], F32, tag="pa", name="psq")
                    for kc in range(NK):
                        nc.tensor.matmul(
                            ps[:],
                            wq[:, kc * D + mc * 128 : kc * D + (mc + 1) * 128],
                            xq_t[:, kc * 512 : (kc + 1) * 512],
                            start=(kc == 0), stop=(kc == NK - 1))
                    nc.scalar.activation(
                        qpT[:, mc * SQ + sc * 512 : mc * SQ + (sc + 1) * 512],
                        ps[:], AF.Identity, bias=bias(_BQ, mc), scale=1.0)

            # vp[s-chunk, d] = xv[kc, s-chunk].T @ Wv[kc, d]  (+bv)
            for sc in range(S // 512):
                xv_t = sb.tile([128, NK * 512], BF, tag="xu", bufs=2, name="xv_t")
                nc.sync.dma_start(
                    out=xv_t[:].rearrange("p (c s) -> p c s", s=512),
                    in_=r6(xv_d)[:, :, sc * 512 : (sc + 1) * 512])
                for m in range(4):
                    srow = sc * 4 + m
                    for n0, nsz in ((0, 512), (512, 256)):
                        ps = pa.tile([128, 512], F32, tag="pa", name="psv")
                        for kc in range(NK):
                            nc.tensor.matmul(
                                ps[:, :nsz],
                                xv_t[:, kc * 512 + m * 128 :
                                     kc * 512 + (m + 1) * 128],
                                wv[:, kc * D + n0 : kc * D + n0 + nsz],
                                start=(kc == 0), stop=(kc == NK - 1))
                        nc.vector.tensor_add(
                            vp[:, srow * D + n0 : srow * D + n0 + nsz],
                            ps[:, :nsz], sp[:, _BV + n0 : _BV + n0 + nsz])

        xqf_sb = sb.tile([128, NK * SQ], F32, tag="xqf", name="xqf_sb")
        nc.sync.dma_start(
            out=xqf_sb[:].rearrange("p (c s) -> p c s", s=SQ), in_=r6(xqf_d))
        if "B" not in PHASES:
            ot = sb.tile([128, 512], F32, tag="sm", bufs=2, name="oend")
            nc.vector.tensor_copy(ot[:], kpT[:, 0:512])
            nc.sync.dma_start(out=r6(out_d)[:, 0, 0:512], in_=ot[:])
            return
        # ---------------- phase B: attention ----------------
        wo = wtile("wo", NK * D)
        nc.sync.dma_start(out=wo[:].rearrange("p (c m) -> p c m", m=D), in_=r6(wo_d))

        with (tc.tile_pool(name="sa_ps", bufs=2, space="PSUM") as sa_ps,
              tc.tile_pool(name="sb_ps", bufs=2, space="PSUM") as sb_ps,
              tc.tile_pool(name="cx_ps", bufs=2, space="PSUM") as cx_ps,
              tc.tile_pool(name="dr_ps", bufs=2, space="PSUM") as dr_ps):
            pending_fin = [None]
            for qc in range(NQ):
                for j in range(HP):
                    hA, hB = 2 * j, 2 * j + 1
                    ctx_p = cx_ps.tile([128, 512], F32, tag="cx", name="ctx_p")
                    # den accumulates at partitions 0 (head A) / 32 (head B);
                    # after r_den extraction the same bank is reused for the
                    # reciprocal broadcast (rb).
                    dn = dr_ps.tile([128, 512], F32, tag="dn", name="dnrb")
                    etile = {}

                    def issue_scores(kc, j=j, qc=qc, etile=etile):
                        psA = sa_ps.tile([128, 512], F32, tag="sa", name="psA")
                        psB = sb_ps.tile([128, 512], F32, tag="sb", name="psB")
                        ko = 0 if B_CONSTW else kc * 128
                        nc.tensor.matmul(
                            psA[:],
                            kpT[0:64, j * S + ko : j * S + ko + 128],
                            qpT[0:64, j * SQ + qc * 512 : j * SQ + (qc + 1) * 512],
                            start=True, stop=True)
                        nc.tensor.matmul(
                            psB[:],
                            kpT[64:128, j * S + ko : j * S + ko + 128],
                            qpT[64:128, j * SQ + qc * 512 : j * SQ + (qc + 1) * 512],
                            start=True, stop=True, skip_group_check=True)
                        eA = sb.tile([128, 512], BF, tag="exA", bufs=3, name="eA")
                        eB = sb.tile([128, 512], BF, tag="exB", bufs=3, name="eB")
                        if B_EXP == "skip":
                            etile[kc] = (e_const, e_const)
                            return
                        nc.scalar.activation(eA[:], psA[:], AF.Exp, scale=SCALE)
                        with nc.allow_low_precision(reason="schraudolph"):
                            nc.vector.tensor_scalar(
                                eB[:].bitcast(I16), psB[:],
                                float(A16), float(B16), OP.mult, OP.add)
                        etile[kc] = (eA, eB)

                    def issue_ctxden(kc, j=j, ctx_p=ctx_p, dn=dn, etile=etile):
                        hA, hB = 2 * j, 2 * j + 1
                        eA, eB = etile.pop(kc)
                        if not B_CTX:
                            return
                        vo = 0 if B_CONSTW else kc * D
                        nc.tensor.matmul(
                            ctx_p[0:64, :],
                            vp[:, vo + hA * 64 : vo + hA * 64 + 64],
                            eA[:, 0:512], start=(kc == 0), stop=(kc == KC - 1),
                            skip_group_check=True)
                        nc.tensor.matmul(
                            ctx_p[64:128, :],
                            vp[:, vo + hB * 64 : vo + hB * 64 + 64],
                            eB[:, 0:512], start=(kc == 0), stop=(kc == KC - 1),
                            skip_group_check=True)
                        if B_DEN:
                            nc.tensor.matmul(
                                dn[0:1, :], ones_bf[:, 0:1], eA[:, 0:512],
                                start=(kc == 0), stop=(kc == KC - 1),
                                skip_group_check=True)
                            nc.tensor.matmul(
                                dn[32:33, :], ones_bf[:, 0:1],
                                eB[:, 0:512],
                                start=(kc == 0), stop=(kc == KC - 1),
                                skip_group_check=True)

                    issue_scores(0)
                    issue_scores(1)
                    for kc in range(KC):
                        if kc + 2 < KC:
                            issue_scores(kc + 2)
                        issue_ctxden(kc)
                        if kc == 3 and pending_fin[0] is not None:
                            pending_fin[0]()
                            pending_fin[0] = None


                    # softmax finalize, deferred into the next head-pair's
                    # kc loop so the rb matmul never stalls the PE stream
                    def finalize(j=j, qc=qc, ctx_p=ctx_p, dn=dn):
                        r_den = sb.tile([64, 512], F32, tag="rd", name="r_den")
                        r_rec = sb.tile([64, 512], FR, tag="rr", name="r_rec")
                        nc.gpsimd.memset(r_den[:, :], 1.0)
                        if B_DEN:
                            nc.vector.tensor_copy(r_den[0:1, :], dn[0:1, :])
                            nc.vector.tensor_copy(r_den[32:33, :], dn[32:33, :])
                        with nc.allow_low_precision(reason="softmax recip"):
                            nc.vector.reciprocal(r_rec[:, :], r_den[:, :])
                        nc.tensor.matmul(dn[:], sel[0:64, :], r_rec[:],
                                         start=True, stop=True)
                        cc = sb.tile([128, 512], F32, tag="cp", bufs=2, name="cc")
                        nc.scalar.activation(cc[:], ctx_p[:], AF.Copy, scale=1.0)
                        with nc.allow_low_precision(reason="bf16 ctx"):
                            nc.vector.tensor_mul(
                                ctxS[:, j * SQ + qc * 512 : j * SQ + (qc + 1) * 512],
                                cc[:], dn[:])
                    if B_DEFER:
                        pending_fin[0] = finalize
                    else:
                        finalize()

        if "C" not in PHASES:
            ot = sb.tile([128, 512], F32, tag="sm", bufs=2, name="oend")
            nc.vector.tensor_copy(ot[:], ctxS[:, 0:512])
            nc.sync.dma_start(out=r6(out_d)[:, 0, 0:512], in_=ot[:])
            return
        if "B" in PHASES and pending_fin[0] is not None:
            pending_fin[0]()
            pending_fin[0] = None
        # ------------ phase C1: out-proj + residual ------------
        x_sb = sb.tile([128, NK * SQ], FR, tag="kpx", name="x_sb")
        with tc.tile_pool(name="pc", bufs=2, space="PSUM") as pc:
            for qc in range(NQ):
                for mc in range(NK):
                    ps = pc.tile([128, 512], F32, tag="pc", name="pso")
                    for kc in range(NK):
                        nc.tensor.matmul(
                            ps[:],
                            wo[:, kc * D + mc * 128 : kc * D + (mc + 1) * 128],
                            ctxS[:, kc * SQ + qc * 512 : kc * SQ + (qc + 1) * 512],
                            start=(kc == 0), stop=(kc == NK - 1))
                    with nc.allow_low_precision(reason="f32r residual"):
                        nc.vector.scalar_tensor_tensor(
                            x_sb[:, mc * SQ + qc * 512 : mc * SQ + (qc + 1) * 512],
                            ps[:], bias(_BO, mc),
                            xqf_sb[:, mc * SQ + qc * 512 : mc * SQ + (qc + 1) * 512],
                            OP.add, OP.add)

        # ---------------- phase C2: LayerNorm ----------------
        hT = sb.tile([128, NK * SQ], BF, tag="hT", name="hT")
        with (tc.tile_pool(name="st_ps", bufs=4, space="PSUM") as st_ps,
              tc.tile_pool(name="ab_ps", bufs=2, space="PSUM") as ab_ps):
            mean_p, var_p = {}, {}
            for qc in range(NQ):
                mean_p[qc] = st_ps.tile([1, 512], F32, tag="st", name=f"mean{qc}")
                var_p[qc] = st_ps.tile([1, 512], F32, tag="st", name=f"var{qc}")
            for kc in range(NK):
                xsq = sb.tile([128, SQ], FR, tag="sq", bufs=2, name="xsq")
                with nc.allow_low_precision(reason="f32r x^2 for LN var"):
                    nc.vector.tensor_mul(
                        xsq[:], x_sb[:, kc * SQ : (kc + 1) * SQ],
                        x_sb[:, kc * SQ : (kc + 1) * SQ])
                for qc in range(NQ):
                    nc.tensor.matmul(
                        mean_p[qc][:], ones_fr[:, 0:1],
                        x_sb[:, kc * SQ + qc * 512 : kc * SQ + (qc + 1) * 512],
                        start=(kc == 0), stop=(kc == NK - 1),
                        skip_group_check=True)
                    nc.tensor.matmul(
                        var_p[qc][:], ones_fr[:, 0:1],
                        xsq[:, qc * 512 : (qc + 1) * 512],
                        start=(kc == 0), stop=(kc == NK - 1),
                        skip_group_check=True)

            for qc in range(NQ):
                mu = sb.tile([1, 512], F32, tag="r1", bufs=2, name="mu")
                e2 = sb.tile([1, 512], F32, tag="r2", bufs=2, name="e2")
                sd = sb.tile([1, 512], F32, tag="r3", bufs=2, name="sd")
                rs = sb.tile([1, 512], FR, tag="r4", bufs=2, name="rs")
                mrs = sb.tile([1, 512], FR, tag="r5", bufs=2, name="mrs")
                nc.vector.tensor_scalar_mul(mu[:], mean_p[qc][:], 1.0 / D)
                nc.vector.tensor_scalar_mul(e2[:], var_p[qc][:], 1.0 / D)
                nc.vector.tensor_mul(sd[:], mu[:], mu[:])
                nc.vector.tensor_sub(e2[:], e2[:], sd[:])        # variance
                nc.vector.tensor_scalar_add(e2[:], e2[:], EPS)
                nc.scalar.activation(sd[:], e2[:], AF.Sqrt)
                with nc.allow_low_precision(reason="f32r LN rows"):
                    nc.vector.reciprocal(rs[:], sd[:])
                    nc.vector.tensor_mul(mrs[:], mu[:].bitcast(FR), rs[:])
                A_p = ab_ps.tile([128, 512], F32, tag="ab", name="A_p")
                B_p = ab_ps.tile([128, 512], F32, tag="ab", name="B_p")
                nc.tensor.matmul(A_p[:], ones_fr[0:1, :], rs[:],
                                 start=True, stop=True)
                nc.tensor.matmul(B_p[:], ones_fr[0:1, :], mrs[:],
                                 start=True, stop=True)
                for kc in range(NK):
                    t1 = sb.tile([128, 512], F32, tag="sm", bufs=2, name="t1")
                    nc.vector.tensor_mul(
                        t1[:],
                        x_sb[:, kc * SQ + qc * 512 : kc * SQ + (qc + 1) * 512],
                        A_p[:])
                    t2 = sb.tile([128, 512], F32, tag="sm", bufs=2, name="t2")
                    nc.vector.tensor_sub(t2[:], t1[:], B_p[:])
                    with nc.allow_low_precision(reason="bf16 hT"):
                        nc.scalar.activation(
                            hT[:, kc * SQ + qc * 512 : kc * SQ + (qc + 1) * 512],
                            t2[:], AF.Identity,
                            bias=bias(_LNB, kc), scale=bias(_LNG, kc))

        if "D" not in PHASES:
            ot = sb.tile([128, 512], F32, tag="sm", bufs=2, name="oend")
            with nc.allow_low_precision(reason="abl"):
                nc.vector.tensor_copy(ot[:], hT[:, 0:512])
            nc.sync.dma_start(out=r6(out_d)[:, 0, 0:512], in_=ot[:])
            return
        # ---------------- phase D: FFN ----------------
        with (tc.tile_pool(name="ff_ps", bufs=6, space="PSUM") as ff_ps,
              tc.tile_pool(name="u_ps", bufs=2, space="PSUM") as u_ps):
            for sc in range(NQ):
                ffp = [ff_ps.tile([128, 512], F32, tag="ff", name=f"ffp{i}")
                       for i in range(NK)]
                for t in range(NT):
                    w1t = wtile(f"w1_{sc}_{t}", NK * DT)
                    nc.sync.dma_start(
                        out=w1t[:].rearrange("p (c m) -> p c m", m=DT),
                        in_=w1_d.rearrange("(c p) (t m) -> p c t m",
                                           p=128, t=NT)[:, :, t, :])
                    w2t = wtile(f"w2_{sc}_{t}", MF * D)
                    nc.sync.dma_start(
                        out=w2t[:].rearrange("p (c m) -> p c m", m=D),
                        in_=w2_d.rearrange("(t c p) m -> p t c m",
                                           p=128, c=MF)[:, t])
                    ut = sb.tile([128, MF * 512], BF, tag="xu", bufs=2, name="ut")
                    for mf in range(MF):
                        up = u_ps.tile([128, 512], F32, tag="up", name="up")
                        for kc in range(NK):
                            nc.tensor.matmul(
                                up[:],
                                w1t[:, kc * DT + mf * 128 : kc * DT + (mf + 1) * 128],
                                hT[:, kc * SQ + sc * 512 : kc * SQ + (sc + 1) * 512],
                                start=(kc == 0), stop=(kc == NK - 1))
                        with nc.allow_low_precision(reason="bf16 u"):
                            nc.scalar.activation(
                                ut[:, mf * 512 : (mf + 1) * 512], up[:],
                                GELU_FUNC, bias=bias(_B1, t * MF + mf),
                                scale=1.0)
                    for md in range(NK):
                        for c in range(MF):
                            nc.tensor.matmul(
                                ffp[md][:],
                                w2t[:, c * D + md * 128 : c * D + (md + 1) * 128],
                                ut[:, c * 512 : (c + 1) * 512],
                                start=(t == 0 and c == 0),
                                stop=(t == NT - 1 and c == MF - 1),
                                skip_group_check=True)
                for md in range(NK):
                    ot = sb.tile([128, 512], F32, tag="sm", bufs=2, name="ot")
                    nc.scalar.activation(ot[:], ffp[md][:], AF.Identity,
                                         bias=bias(_B2, md), scale=1.0)
                    nc.sync.dma_start(
                        out=r6(out_d)[:, md, sc * 512 : (sc + 1) * 512],
                        in_=ot[:])


def _build(reps=1):
    nc = bacc.Bacc("TRN2", target_bir_lowering=False, debug=False,
                   num_devices=N_CORES)
    io = {
        "xqb": nc.dram_tensor("xqb", [D, SQ], BF, kind="ExternalInput").ap(),
        "xqf": nc.dram_tensor("xqf", [D, SQ], F32, kind="ExternalInput").ap(),
        "xk": nc.dram_tensor("xk", [D, S], BF, kind="ExternalInput").ap(),
        "xv": nc.dram_tensor("xv", [D, S], BF, kind="ExternalInput").ap(),
        "wq": nc.dram_tensor("wq", [D, D], BF, kind="ExternalInput").ap(),
        "wk": nc.dram_tensor("wk", [D, D], BF, kind="ExternalInput").ap(),
        "wv": nc.dram_tensor("wv", [D, D], BF, kind="ExternalInput").ap(),
        "wo": nc.dram_tensor("wo", [D, D], BF, kind="ExternalInput").ap(),
        "w1": nc.dram_tensor("w1", [D, DF], BF, kind="ExternalInput").ap(),
        "w2": nc.dram_tensor("w2", [DF, D], BF, kind="ExternalInput").ap(),
        "sp": nc.dram_tensor("sp", [128, 828], F32, kind="ExternalInput").ap(),
        "ones_fr": nc.dram_tensor("ones_fr", [128, 128], FR,
                                  kind="ExternalInput").ap(),
        "ones_bf": nc.dram_tensor("ones_bf", [128, 1], BF,
                                  kind="ExternalInput").ap(),
        "sel": nc.dram_tensor("sel", [128, 128], FR, kind="ExternalInput").ap(),
        "out": nc.dram_tensor("out", [D, SQ], F32, kind="ExternalOutput").ap(),
    }
    with tile.TileContext(nc) as tc:
        if reps == 1:
            _body(nc, tc, io)
        else:
            with tc.For_i(0, reps, 1):
                _body(nc, tc, io)
    nc.compile()
    return nc


_NC = None


def _get_nc():
    global _NC
    if _NC is None:
        _NC = _build()
    return _NC


def _sel_matrix():
    sel = np.zeros((128, 128), np.float32)
    sel[0, 0:64] = 1.0
    sel[32, 64:128] = 1.0
    return sel


def make_in_maps(inputs):
    """Shard + lay out the full inputs for the 8 cores (numpy only)."""
    f = lambda k: np.asarray(inputs[k], np.float32)
    bf = lambda a: np.ascontiguousarray(a).astype(ml_dtypes.bfloat16)
    Q, K, V = f("Q"), f("K"), f("V")
    sp = np.zeros((128, 828), np.float32)
    for idx, key in ((_BQ, "bq"), (_BK, "bk"), (_BO, "bo"), (_B2, "b2"),
                     (_LNG, "ln_g"), (_LNB, "ln_b")):
        sp[:, idx : idx + NK] = f(key).reshape(NK, 128).T
    sp[:, _B1 : _B1 + NF] = f("b1").reshape(NF, 128).T
    sp[:, _BV : _BV + D] = np.broadcast_to(f("bv"), (128, D))
    shared = {
        "wq": bf(f("Wq")), "wk": bf(f("Wk")), "wv": bf(f("Wv")),
        "wo": bf(f("Wo")), "w1": bf(f("W1")), "w2": bf(f("W2")), "sp": sp,
        "ones_fr": np.ones((128, 128), np.float32),
        "ones_bf": np.ones((128, 1), ml_dtypes.bfloat16),
        "sel": _sel_matrix(),
    }
    in_maps = []
    for c in range(N_CORES):
        b, half = divmod(c, 2)
        r0 = half * SQ
        xqf = np.ascontiguousarray(Q[b, r0 : r0 + SQ, :].T)
        in_maps.append(dict(
            shared,
            xqb=xqf.astype(ml_dtypes.bfloat16),
            xqf=xqf,
            xk=bf(K[b].T),
            xv=bf(V[b].T),
        ))
    return in_maps


def assemble(results):
    out = np.empty((B, S, D), np.float32)
    for c in range(N_CORES):
        b, half = divmod(c, 2)
        r0 = half * SQ
        out[b, r0 : r0 + SQ, :] = results[c]["out"].T
    return out


def kernel(**inputs):
    nc = _get_nc()
    res = run_bass_kernel_spmd(nc, make_in_maps(inputs), list(range(N_CORES)))
    return assemble(res.results)
